# revision 8
# baseline (speedup 1.0000x reference)
"""Trainium2 Bass kernel for nn_Attention_73770358276185.

Per-batch computation (B=8, one batch per NeuronCore, data-parallel):
    f = gelu(BN(Wf @ q + bf))            [64, 4096]
    g = gelu(BN(Wg @ k + bg))            [64, 4096]
    h = gelu(BN(Wh @ k + bh))            [256, 4096]
    s[i,j] = sum_l g[l,i] f[l,j]         [4096, 4096]
    beta = softmax_j(s)
    o[i,c] = sum_j beta[i,j] h[c,j]
    out = gamma * o.T + q

Layout tricks:
  * sT[j,i] (j on partitions) so the softmax contraction (over j) is the
    matmul-partition dim for the second matmul.  softmax runs without
    max-subtraction (s_max ~ 69, exp stays in fp32 range); the row-sum is
    obtained free by augmenting hT with a ones column.
  * mm1 has true K=64, so f and g are stored DUPLICATED across the two
    partition halves (via column-duplicated projection weights, which costs
    nothing) and mm1 runs as a row-tiled pair of K=64 matmuls at PE tile
    positions (0,0)/(64,0) — concurrent on HW, ~2x mm1 throughput.
  * q is loaded once, pre-rounded to tf32 on the host: the same bytes serve
    the f-projection matmul (as float32r) and the residual add (bitcast to
    float32).  Halves HBM load traffic on the critical path.
  * All big matmuls run in float32r (TF32, 1 cycle/row on the PE).
"""
import sys

for _p in ("/opt/trn_rl_repo", "/root/.axon_site/_ro/trn_rl_repo"):
    if _p not in sys.path:
        sys.path.insert(0, _p)

import numpy as np

import concourse.bacc as bacc
import concourse.tile as tile
import concourse.mybir as mybir
from concourse.bass_utils import run_bass_kernel_spmd

P = 128
B = 8
N = 4096          # sequence positions
C1 = 256          # dim1 (q channels / h channels)
C2 = 128          # dim2 (k channels)
L = 64            # layer = dim1 // 4 (f/g channels)
EPS = 1e-5

NJB = N // P      # 32 j-blocks
NIC = 8           # i chunks
IC = N // NIC     # 512 columns per i chunk
JG = 2            # j-blocks per exp group
NGRP = NJB // JG  # 16 groups
HST = 258         # h_aug row stride (256 ch + ones col + pad; even for f32r)

F32 = mybir.dt.float32
F32R = mybir.dt.float32r
AF = mybir.ActivationFunctionType
MUL = mybir.AluOpType.mult

_BUILT = None  # (nc) cache — the program is input-value independent


def _round_tf32(x):
    """Round fp32 to float32r (drop 12 mantissa bits, round-to-nearest)."""
    v = np.ascontiguousarray(x, dtype=np.float32).view(np.uint32).astype(np.uint64)
    half = np.uint64(0x7FF)
    lsb = (v >> np.uint64(12)) & np.uint64(1)
    v = (v + half + lsb) & np.uint64(0xFFFFF000)
    return v.astype(np.uint32).view(np.float32)


def _build(repeat=1, loads_in_loop=False):
    nc = bacc.Bacc("TRN2", target_bir_lowering=False, debug=False)

    # q pre-rounded to tf32 on host; serves matmul (f32r) and residual (f32).
    qr = nc.dram_tensor("qr", [C1, N], F32R, kind="ExternalInput")
    k2r = nc.dram_tensor("k2r", [C2, N], F32R, kind="ExternalInput")
    wfd = nc.dram_tensor("wfd", [C1, P], F32R, kind="ExternalInput")   # dup cols
    wgd = nc.dram_tensor("wgd", [C2, P], F32R, kind="ExternalInput")   # dup cols
    whT = nc.dram_tensor("whT", [C2, C1], F32R, kind="ExternalInput")
    dfd = nc.dram_tensor("dfd", [P, 1], F32, kind="ExternalInput")     # dup bias
    dgd = nc.dram_tensor("dgd", [P, 1], F32, kind="ExternalInput")
    dhbc = nc.dram_tensor("dhbc", [P, C1], F32, kind="ExternalInput")
    gmb = nc.dram_tensor("gmb", [P, 1], F32, kind="ExternalInput")
    ident = nc.dram_tensor("ident", [P, P], F32R, kind="ExternalInput")
    o_out = nc.dram_tensor("o_out", [C1, N], F32, kind="ExternalOutput")

    with tile.TileContext(nc) as tc:
        with (
            tc.tile_pool(name="const", bufs=1) as cp,
            tc.tile_pool(name="acc", bufs=4, space="PSUM") as accp,
            tc.tile_pool(name="sT", bufs=2, space="PSUM") as sTp,
            tc.tile_pool(name="ex", bufs=3) as exp_,
            tc.tile_pool(name="osc", bufs=4) as oscp,
            tc.tile_pool(name="rin", bufs=4) as rinp,
            tc.tile_pool(name="outst", bufs=4) as outp,
        ):
            if not loads_in_loop:
                env0 = _emit_loads(nc, tc, locals())
            else:
                env0 = None

            import contextlib
            loop_cm = tc.For_i(0, repeat, 1) if repeat > 1 else contextlib.nullcontext()
            with loop_cm:
                _env = dict(locals())
                if loads_in_loop:
                    env0 = _emit_loads(nc, tc, _env)
                _env.update(env0)
                _emit_body(nc, tc, _env)

    nc.finalize()
    return nc


def _emit_loads(nc, tc, env):
    cp = env["cp"]
    k2r = env["k2r"]; qr = env["qr"]
    wfd = env["wfd"]; wgd = env["wgd"]; whT = env["whT"]
    dfd = env["dfd"]; dgd = env["dgd"]; dhbc = env["dhbc"]; gmb = env["gmb"]
    ident = env["ident"]
    # Load order matters: first k chunk + g/h weights (unblocks g/h
    # projections at ~2.5us), rest of k, then f weights and the q halves in
    # column chunks (f projection streams behind the loads).  gm/idt are only
    # needed at the first epilogue (~40us) so they go last.
    k_sb = cp.tile([C2, N], F32R, tag="k")
    nc.sync.dma_start(k_sb[:, 0:1024], k2r[:, 0:1024])
    wg = cp.tile([C2, P], F32R, tag="wg")
    nc.sync.dma_start(wg[:], wgd[:, :])
    wh = cp.tile([C2, C1], F32R, tag="wh")
    nc.sync.dma_start(wh[:], whT[:, :])
    dgt = cp.tile([P, 1], F32, tag="dg")
    nc.sync.dma_start(dgt[:], dgd[:, :])
    dht = cp.tile([P, C1], F32, tag="dh")
    nc.sync.dma_start(dht[:], dhbc[:, :])
    for c in range(1, 4):
        nc.sync.dma_start(k_sb[:, c * 1024:(c + 1) * 1024],
                          k2r[:, c * 1024:(c + 1) * 1024])
    wf = [cp.tile([P, P], F32R, tag=f"wf{i}", name=f"wf{i}") for i in range(2)]
    for i in range(2):
        nc.sync.dma_start(wf[i][:], wfd[i * P:(i + 1) * P, :])
    dft = cp.tile([P, 1], F32, tag="df")
    nc.sync.dma_start(dft[:], dfd[:, :])
    q_sb = [cp.tile([P, N], F32R, tag=f"q{cb}", name=f"q{cb}") for cb in range(2)]
    for c in range(2):
        for cb in range(2):
            nc.sync.dma_start(q_sb[cb][:, c * 2048:(c + 1) * 2048],
                              qr[cb * P:(cb + 1) * P, c * 2048:(c + 1) * 2048])
    gm = cp.tile([P, 1], F32, tag="gm")
    nc.sync.dma_start(gm[:], gmb[:, :])
    idt = cp.tile([P, P], F32R, tag="id")
    nc.sync.dma_start(idt[:], ident[:, :])

    return dict(k_sb=k_sb, q_sb=q_sb, wf=wf, wg=wg, wh=wh,
                dft=dft, dgt=dgt, dht=dht, gm=gm, idt=idt)


def _emit_body(nc, tc, env):
    accp = env["accp"]; sTp = env["sTp"]; exp_ = env["exp_"]
    oscp = env["oscp"]; rinp = env["rinp"]; outp = env["outp"]; cp = env["cp"]
    k_sb = env["k_sb"]; q_sb = env["q_sb"]
    wf = env["wf"]; wg = env["wg"]; wh = env["wh"]
    dft = env["dft"]; dgt = env["dgt"]; dht = env["dht"]; gm = env["gm"]
    idt = env["idt"]; o_out = env["o_out"]

    # f/g live duplicated across partition halves (rows 0:64 == rows 64:128),
    # produced directly by the column-duplicated projection weights.
    f_sb = cp.tile([P, N], F32R, tag="f")
    g_sb = cp.tile([P, N], F32R, tag="g")
    h_aug = cp.tile([P, NJB * HST], F32R, tag="h")
    # ones columns of h_aug (col 256 of each 258-block): one strided memset
    h3f = h_aug[:].bitcast(F32).rearrange("p (b c) -> p b c", c=HST)
    h3 = h_aug[:].rearrange("p (b c) -> p b c", c=HST)
    nc.vector.memset(h3f[:, :, C1:C1 + 1], 1.0)
    nc.vector.memset(h3f[:, :, C1 + 1:HST], 0.0)

    # ---- g, h, f projections (g/h first: they only depend on k) ----------
    for n in range(NIC):
        ps = accp.tile([P, IC], F32, tag="acc", name="gps")
        nc.tensor.matmul(ps[:], wg[:], k_sb[:, n * IC:(n + 1) * IC],
                         start=True, stop=True)
        nc.scalar.activation(g_sb[:, n * IC:(n + 1) * IC], ps[:],
                             AF.Gelu, bias=dgt[:])
    for jp in range(NJB // 2):
        ps = accp.tile([P, 2 * C1], F32, tag="acc", name="hps")
        for t in range(2):
            jb = 2 * jp + t
            nc.tensor.matmul(ps[:, t * C1:(t + 1) * C1],
                             k_sb[:, jb * P:(jb + 1) * P], wh[:],
                             start=True, stop=True)
        nc.vector.tensor_add(ps[:, 0:C1], ps[:, 0:C1], dht[:])
        nc.vector.tensor_add(ps[:, C1:2 * C1], ps[:, C1:2 * C1], dht[:])
        nc.scalar.activation(h3[:, 2 * jp:2 * jp + 2, 0:C1], ps[:], AF.Gelu)
    for n in range(NIC):
        ps = accp.tile([P, IC], F32, tag="acc", name="fps")
        nc.tensor.matmul(ps[:], wf[0][:], q_sb[0][:, n * IC:(n + 1) * IC],
                         start=True, stop=False)
        nc.tensor.matmul(ps[:], wf[1][:], q_sb[1][:, n * IC:(n + 1) * IC],
                         start=False, stop=True)
        nc.scalar.activation(f_sb[:, n * IC:(n + 1) * IC], ps[:],
                             AF.Gelu, bias=dft[:])

    # ---- attention main loop (software-pipelined emission) ---------------
    o_augs = {}

    def emit_mm1(ic, grp):
        # Row-tiled pair: K=64 matmuls at PE rows 0-63 / 64-127, concurrent.
        sT = sTp.tile([P, JG * IC], F32, tag="sT", name="sT")
        for t in range(JG):
            jb = grp * JG + t
            h0 = t * L  # 0 or 64: partition half (auto tile_position)
            nc.tensor.matmul(sT[:, t * IC:(t + 1) * IC],
                             f_sb[h0:h0 + L, jb * P:(jb + 1) * P],
                             g_sb[h0:h0 + L, ic * IC:(ic + 1) * IC],
                             start=True, stop=True)
        ex = exp_.tile([P, JG * IC], F32R, tag="ex", name="ex")
        nc.scalar.activation(ex[:], sT[:], AF.Exp)
        return ex

    def emit_mm2(ic, grp, ex):
        if grp == 0:
            o_augs[ic] = [
                accp.tile([P, HST], F32, tag="acc", name=f"oaug{ib}")
                for ib in range(4)]
        o_aug = o_augs[ic]
        for t in range(JG):
            jb = grp * JG + t
            for ib in range(4):
                nc.tensor.matmul(
                    o_aug[ib][:],
                    ex[:, t * IC + ib * P:t * IC + (ib + 1) * P],
                    h_aug[:, jb * HST:(jb + 1) * HST],
                    start=(grp == 0 and t == 0),
                    stop=(grp == NGRP - 1 and t == JG - 1))

    def emit_epilogue(ic):
        o_aug = o_augs.pop(ic)
        ost = [outp.tile([P, IC], F32, tag=f"ost{cb}", name=f"ost{cb}")
               for cb in range(2)]
        oscs = []
        for ib in range(4):
            rv = rinp.tile([P, 1], F32, tag="rin", name="rv")
            nc.vector.reciprocal(rv[:], o_aug[ib][:, C1:C1 + 1])
            osc = oscp.tile([P, C1], F32R, tag="osc", name="osc")
            nc.vector.tensor_scalar(osc[:], o_aug[ib][:, 0:C1], rv[:],
                                    gm[:], op0=MUL, op1=MUL)
            oscs.append(osc)
        for ib in range(4):
            for cb in range(2):
                oT = accp.tile([P, P], F32R, tag="acc", name="oT")
                nc.tensor.transpose(oT[:], oscs[ib][:, cb * P:(cb + 1) * P],
                                    idt[:])
                nc.vector.tensor_add(
                    ost[cb][:, ib * P:(ib + 1) * P], oT[:].bitcast(F32),
                    q_sb[cb][:, ic * IC + ib * P:ic * IC + (ib + 1) * P]
                    .bitcast(F32))
        for cb in range(2):
            nc.sync.dma_start(
                o_out[cb * P:(cb + 1) * P, ic * IC:(ic + 1) * IC],
                ost[cb][:])

    groups = [(ic, grp) for ic in range(NIC) for grp in range(NGRP)]
    pending = None  # (ic, grp, ex) whose mm2 is not yet emitted
    for (ic, grp) in groups:
        ex = emit_mm1(ic, grp)
        if pending is not None:
            pic, pgrp, pex = pending
            emit_mm2(pic, pgrp, pex)
            if pgrp == NGRP - 1:
                emit_epilogue(pic)
        pending = (ic, grp, ex)
    pic, pgrp, pex = pending
    emit_mm2(pic, pgrp, pex)
    emit_epilogue(pic)


def _preprocess(inputs):
    """Fold conv bias + BN into effective weights/biases, per-core input maps."""
    f32 = np.float32
    q = np.ascontiguousarray(inputs["q"], dtype=f32)[..., 0]   # [B, 256, N]
    k = np.ascontiguousarray(inputs["k"], dtype=f32)[..., 0]   # [B, 128, N]

    def fold(W, b, scale, bias, mean, var):
        inv = (np.asarray(scale, f32) /
               np.sqrt(np.asarray(var, f32) + f32(EPS))).astype(f32)
        W_eff = (inv[:, None] * np.asarray(W, f32)).astype(f32)
        delta = ((np.asarray(b, f32) - np.asarray(mean, f32)) * inv
                 + np.asarray(bias, f32)).astype(f32)
        return W_eff, delta

    Wf_e, d_f = fold(inputs["Wf"], inputs["bf"], inputs["fs"], inputs["fb"],
                     inputs["fm"], inputs["fv"])
    Wg_e, d_g = fold(inputs["Wg"], inputs["bg"], inputs["gs"], inputs["gb"],
                     inputs["gm"], inputs["gv"])
    Wh_e, d_h = fold(inputs["Wh"], inputs["bh"], inputs["hs"], inputs["hb"],
                     inputs["hm"], inputs["hv"])

    gamma = f32(np.asarray(inputs["gamma"], f32).reshape(-1)[0])
    wfT = Wf_e.T                                   # [256, 64]
    wgT = Wg_e.T                                   # [128, 64]
    shared = {
        "wfd": _round_tf32(np.concatenate([wfT, wfT], axis=1)),  # [256, 128]
        "wgd": _round_tf32(np.concatenate([wgT, wgT], axis=1)),  # [128, 128]
        "whT": _round_tf32(Wh_e.T),                              # [128, 256]
        "dfd": np.tile(d_f.reshape(L, 1), (2, 1)),               # [128, 1]
        "dgd": np.tile(d_g.reshape(L, 1), (2, 1)),
        "dhbc": np.broadcast_to(d_h, (P, C1)).copy(),
        "gmb": np.full((P, 1), gamma, f32),
        "ident": np.eye(P, dtype=f32),
    }
    in_maps = []
    for b_ in range(B):
        m = dict(shared)
        m["qr"] = _round_tf32(q[b_])
        m["k2r"] = _round_tf32(k[b_])
        in_maps.append(m)
    return in_maps


def _get_nc():
    global _BUILT
    if _BUILT is None:
        _BUILT = _build()
    return _BUILT


def kernel(**inputs):
    nc = _get_nc()
    in_maps = _preprocess(inputs)
    res = run_bass_kernel_spmd(nc, in_maps, core_ids=list(range(B)))
    out = np.stack([res.results[i]["o_out"] for i in range(B)])
    return out[..., None].astype(np.float32)


if __name__ == "__main__":
    rng = np.random.default_rng(0)
    fake = {
        "q": rng.standard_normal((B, C1, N, 1), dtype=np.float32),
        "k": rng.standard_normal((B, C2, N, 1), dtype=np.float32),
        "Wf": rng.standard_normal((L, C1), dtype=np.float32) * 0.06,
        "bf": rng.standard_normal(L, dtype=np.float32) * 0.01,
        "fs": rng.random(L, dtype=np.float32) + 0.5,
        "fb": rng.standard_normal(L, dtype=np.float32) * 0.1,
        "fm": rng.standard_normal(L, dtype=np.float32) * 0.1,
        "fv": rng.random(L, dtype=np.float32) + 0.5,
        "Wg": rng.standard_normal((L, C2), dtype=np.float32) * 0.09,
        "bg": rng.standard_normal(L, dtype=np.float32) * 0.01,
        "gs": rng.random(L, dtype=np.float32) + 0.5,
        "gb": rng.standard_normal(L, dtype=np.float32) * 0.1,
        "gm": rng.standard_normal(L, dtype=np.float32) * 0.1,
        "gv": rng.random(L, dtype=np.float32) + 0.5,
        "Wh": rng.standard_normal((C1, C2), dtype=np.float32) * 0.09,
        "bh": rng.standard_normal(C1, dtype=np.float32) * 0.01,
        "hs": rng.random(C1, dtype=np.float32) + 0.5,
        "hb": rng.standard_normal(C1, dtype=np.float32) * 0.1,
        "hm": rng.standard_normal(C1, dtype=np.float32) * 0.1,
        "hv": rng.random(C1, dtype=np.float32) + 0.5,
        "gamma": np.array([-1.1], dtype=np.float32),
    }
    out = kernel(**fake)
    print("out", out.shape, out.dtype, float(np.abs(out).max()))


# revision 10
# speedup vs baseline: 1.1311x; 1.1311x over previous
"""Trainium2 Bass kernel for nn_Attention_73770358276185.

Per-batch computation (B=8, one batch per NeuronCore, data-parallel):
    f = gelu(BN(Wf @ q + bf))            [64, 4096]
    g = gelu(BN(Wg @ k + bg))            [64, 4096]
    h = gelu(BN(Wh @ k + bh))            [256, 4096]
    s[i,j] = sum_l g[l,i] f[l,j]         [4096, 4096]
    beta = softmax_j(s)
    o[i,c] = sum_j beta[i,j] h[c,j]
    out = gamma * o.T + q

Layout tricks:
  * sT[j,i] (j on partitions) so the softmax contraction (over j) is the
    matmul-partition dim for the second matmul.  softmax runs without
    max-subtraction (s_max ~ 69, exp stays in fp32 range); the row-sum is
    obtained free by augmenting hT with a ones column.
  * mm1 has true K=64, so f and g are stored DUPLICATED across the two
    partition halves (via column-duplicated projection weights, which costs
    nothing) and mm1 runs as a row-tiled pair of K=64 matmuls at PE tile
    positions (0,0)/(64,0) — concurrent on HW, ~2x mm1 throughput.
  * q is loaded once, pre-rounded to tf32 on the host: the same bytes serve
    the f-projection matmul (as float32r) and the residual add (bitcast to
    float32).  Halves HBM load traffic on the critical path.
  * All big matmuls run in float32r (TF32, 1 cycle/row on the PE).
"""
import sys

for _p in ("/opt/trn_rl_repo", "/root/.axon_site/_ro/trn_rl_repo"):
    if _p not in sys.path:
        sys.path.insert(0, _p)

import numpy as np

import concourse.bacc as bacc
import concourse.tile as tile
import concourse.mybir as mybir
from concourse.bass_utils import run_bass_kernel_spmd

P = 128
B = 8
N = 4096          # sequence positions
C1 = 256          # dim1 (q channels / h channels)
C2 = 128          # dim2 (k channels)
L = 64            # layer = dim1 // 4 (f/g channels)
EPS = 1e-5

NJB = N // P      # 32 j-blocks
NIC = 8           # i chunks
IC = N // NIC     # 512 columns per i chunk
JG = 2            # j-blocks per exp group
NGRP = NJB // JG  # 16 groups
HST = 258         # h_aug row stride (256 ch + ones col + pad; even for f32r)

F32 = mybir.dt.float32
F32R = mybir.dt.float32r
BF = mybir.dt.bfloat16
AF = mybir.ActivationFunctionType
MUL = mybir.AluOpType.mult

_BUILT = None  # (nc) cache — the program is input-value independent


def _round_tf32(x):
    """Round fp32 to float32r (drop 12 mantissa bits, round-to-nearest)."""
    v = np.ascontiguousarray(x, dtype=np.float32).view(np.uint32).astype(np.uint64)
    half = np.uint64(0x7FF)
    lsb = (v >> np.uint64(12)) & np.uint64(1)
    v = (v + half + lsb) & np.uint64(0xFFFFF000)
    return v.astype(np.uint32).view(np.float32)


def _to_bf16(x):
    """fp32 -> bf16 (round-to-nearest-even), as uint16-backed ml_dtypes array."""
    import ml_dtypes
    return np.asarray(x, dtype=np.float32).astype(ml_dtypes.bfloat16)


def _build(repeat=1, loads_in_loop=False):
    nc = bacc.Bacc("TRN2", target_bir_lowering=False, debug=False)

    # q pre-rounded to tf32 on host; serves matmul (f32r) and residual (f32).
    qr = nc.dram_tensor("qr", [C1, N], F32R, kind="ExternalInput")
    k2r = nc.dram_tensor("k2r", [C2, N], F32R, kind="ExternalInput")
    wfd = nc.dram_tensor("wfd", [C1, P], F32R, kind="ExternalInput")   # dup cols
    wgd = nc.dram_tensor("wgd", [C2, P], F32R, kind="ExternalInput")   # dup cols
    whT = nc.dram_tensor("whT", [C2, C1], F32R, kind="ExternalInput")
    dfd = nc.dram_tensor("dfd", [P, 1], F32, kind="ExternalInput")     # dup bias
    dgd = nc.dram_tensor("dgd", [P, 1], F32, kind="ExternalInput")
    dhbc = nc.dram_tensor("dhbc", [P, C1], F32, kind="ExternalInput")
    gmb = nc.dram_tensor("gmb", [P, 1], F32, kind="ExternalInput")
    ident = nc.dram_tensor("ident", [P, P], BF, kind="ExternalInput")
    o_out = nc.dram_tensor("o_out", [C1, N], F32, kind="ExternalOutput")

    with tile.TileContext(nc) as tc:
        with (
            tc.tile_pool(name="const", bufs=1) as cp,
            tc.tile_pool(name="acc", bufs=4, space="PSUM") as accp,
            tc.tile_pool(name="sT", bufs=2, space="PSUM") as sTp,
            tc.tile_pool(name="ex", bufs=4) as exp_,
            tc.tile_pool(name="osc", bufs=4) as oscp,
            tc.tile_pool(name="rin", bufs=4) as rinp,
            tc.tile_pool(name="outst", bufs=4) as outp,
        ):
            if not loads_in_loop:
                env0 = _emit_loads(nc, tc, locals())
            else:
                env0 = None

            import contextlib
            loop_cm = tc.For_i(0, repeat, 1) if repeat > 1 else contextlib.nullcontext()
            with loop_cm:
                _env = dict(locals())
                if loads_in_loop:
                    env0 = _emit_loads(nc, tc, _env)
                _env.update(env0)
                _emit_body(nc, tc, _env)

    nc.finalize()
    return nc


def _emit_loads(nc, tc, env):
    cp = env["cp"]
    k2r = env["k2r"]; qr = env["qr"]
    wfd = env["wfd"]; wgd = env["wgd"]; whT = env["whT"]
    dfd = env["dfd"]; dgd = env["dgd"]; dhbc = env["dhbc"]; gmb = env["gmb"]
    ident = env["ident"]
    # Load order matters: first k chunk + g/h weights (unblocks g/h
    # projections at ~2.5us), rest of k, then f weights and the q halves in
    # column chunks (f projection streams behind the loads).  gm/idt are only
    # needed at the first epilogue (~40us) so they go last.
    k_sb = cp.tile([C2, N], F32R, tag="k")
    nc.sync.dma_start(k_sb[:, 0:1024], k2r[:, 0:1024])
    wg = cp.tile([C2, P], F32R, tag="wg")
    nc.sync.dma_start(wg[:], wgd[:, :])
    wh = cp.tile([C2, C1], F32R, tag="wh")
    nc.sync.dma_start(wh[:], whT[:, :])
    dgt = cp.tile([P, 1], F32, tag="dg")
    nc.sync.dma_start(dgt[:], dgd[:, :])
    dht = cp.tile([P, C1], F32, tag="dh")
    nc.sync.dma_start(dht[:], dhbc[:, :])
    for c in range(1, 4):
        nc.sync.dma_start(k_sb[:, c * 1024:(c + 1) * 1024],
                          k2r[:, c * 1024:(c + 1) * 1024])
    wf = [cp.tile([P, P], F32R, tag=f"wf{i}", name=f"wf{i}") for i in range(2)]
    for i in range(2):
        nc.sync.dma_start(wf[i][:], wfd[i * P:(i + 1) * P, :])
    dft = cp.tile([P, 1], F32, tag="df")
    nc.sync.dma_start(dft[:], dfd[:, :])
    q_sb = [cp.tile([P, N], F32R, tag=f"q{cb}", name=f"q{cb}") for cb in range(2)]
    for c in range(2):
        for cb in range(2):
            nc.sync.dma_start(q_sb[cb][:, c * 2048:(c + 1) * 2048],
                              qr[cb * P:(cb + 1) * P, c * 2048:(c + 1) * 2048])
    gm = cp.tile([P, 1], F32, tag="gm")
    nc.sync.dma_start(gm[:], gmb[:, :])
    idt = cp.tile([P, P], BF, tag="id")
    nc.sync.dma_start(idt[:], ident[:, :])

    return dict(k_sb=k_sb, q_sb=q_sb, wf=wf, wg=wg, wh=wh,
                dft=dft, dgt=dgt, dht=dht, gm=gm, idt=idt)


def _emit_body(nc, tc, env):
    accp = env["accp"]; sTp = env["sTp"]; exp_ = env["exp_"]
    oscp = env["oscp"]; rinp = env["rinp"]; outp = env["outp"]; cp = env["cp"]
    k_sb = env["k_sb"]; q_sb = env["q_sb"]
    wf = env["wf"]; wg = env["wg"]; wh = env["wh"]
    dft = env["dft"]; dgt = env["dgt"]; dht = env["dht"]; gm = env["gm"]
    idt = env["idt"]; o_out = env["o_out"]

    # f/g live duplicated across partition halves (rows 0:64 == rows 64:128),
    # produced directly by the column-duplicated projection weights.
    f_sb = cp.tile([P, N], BF, tag="f")
    g_sb = cp.tile([P, N], BF, tag="g")
    h_aug = cp.tile([P, NJB * HST], BF, tag="h")
    # ones columns of h_aug (col 256 of each 258-block): one strided memset
    h3 = h_aug[:].rearrange("p (b c) -> p b c", c=HST)
    nc.vector.memset(h3[:, :, C1:C1 + 1], 1.0)
    nc.vector.memset(h3[:, :, C1 + 1:HST], 0.0)

    # ---- g, h, f projections (g/h first: they only depend on k) ----------
    for n in range(NIC):
        ps = accp.tile([P, IC], F32, tag="acc", name="gps")
        nc.tensor.matmul(ps[:], wg[:], k_sb[:, n * IC:(n + 1) * IC],
                         start=True, stop=True)
        nc.scalar.activation(g_sb[:, n * IC:(n + 1) * IC], ps[:],
                             AF.Gelu, bias=dgt[:])
    for jp in range(NJB // 2):
        ps = accp.tile([P, 2 * C1], F32, tag="acc", name="hps")
        for t in range(2):
            jb = 2 * jp + t
            nc.tensor.matmul(ps[:, t * C1:(t + 1) * C1],
                             k_sb[:, jb * P:(jb + 1) * P], wh[:],
                             start=True, stop=True)
        nc.vector.tensor_add(ps[:, 0:C1], ps[:, 0:C1], dht[:])
        nc.vector.tensor_add(ps[:, C1:2 * C1], ps[:, C1:2 * C1], dht[:])
        nc.scalar.activation(h3[:, 2 * jp:2 * jp + 2, 0:C1], ps[:], AF.Gelu)
    for n in range(NIC):
        ps = accp.tile([P, IC], F32, tag="acc", name="fps")
        nc.tensor.matmul(ps[:], wf[0][:], q_sb[0][:, n * IC:(n + 1) * IC],
                         start=True, stop=False)
        nc.tensor.matmul(ps[:], wf[1][:], q_sb[1][:, n * IC:(n + 1) * IC],
                         start=False, stop=True)
        nc.scalar.activation(f_sb[:, n * IC:(n + 1) * IC], ps[:],
                             AF.Gelu, bias=dft[:])

    # ---- attention main loop (software-pipelined emission) ---------------
    o_augs = {}

    def emit_mm1(ic, grp):
        # Row-tiled pair: K=64 matmuls at PE rows 0-63 / 64-127, concurrent.
        sT = sTp.tile([P, JG * IC], F32, tag="sT", name="sT")
        for t in range(JG):
            jb = grp * JG + t
            h0 = t * L  # 0 or 64: partition half (auto tile_position)
            nc.tensor.matmul(sT[:, t * IC:(t + 1) * IC],
                             f_sb[h0:h0 + L, jb * P:(jb + 1) * P],
                             g_sb[h0:h0 + L, ic * IC:(ic + 1) * IC],
                             start=True, stop=True)
        ex = exp_.tile([P, JG * IC], BF, tag="ex", name="ex")
        nc.scalar.activation(ex[:], sT[:], AF.Exp)
        return ex

    def emit_mm2(ic, grp, ex):
        if grp == 0:
            o_augs[ic] = [
                accp.tile([P, HST], F32, tag="acc", name=f"oaug{ib}")
                for ib in range(4)]
        o_aug = o_augs[ic]
        for t in range(JG):
            jb = grp * JG + t
            for ib in range(4):
                nc.tensor.matmul(
                    o_aug[ib][:],
                    ex[:, t * IC + ib * P:t * IC + (ib + 1) * P],
                    h_aug[:, jb * HST:(jb + 1) * HST],
                    start=(grp == 0 and t == 0),
                    stop=(grp == NGRP - 1 and t == JG - 1))

    def emit_epilogue(ic):
        o_aug = o_augs.pop(ic)
        ost = [outp.tile([P, IC], F32, tag=f"ost{cb}", name=f"ost{cb}")
               for cb in range(2)]
        oscs = []
        for ib in range(4):
            rv = rinp.tile([P, 1], F32, tag="rin", name="rv")
            nc.vector.reciprocal(rv[:], o_aug[ib][:, C1:C1 + 1])
            osc = oscp.tile([P, C1], BF, tag="osc", name="osc")
            nc.vector.tensor_scalar(osc[:], o_aug[ib][:, 0:C1], rv[:],
                                    gm[:], op0=MUL, op1=MUL)
            oscs.append(osc)
        for ib in range(4):
            for cb in range(2):
                oT = accp.tile([P, P], BF, tag="acc", name="oT")
                nc.tensor.transpose(oT[:], oscs[ib][:, cb * P:(cb + 1) * P],
                                    idt[:])
                nc.vector.tensor_add(
                    ost[cb][:, ib * P:(ib + 1) * P], oT[:],
                    q_sb[cb][:, ic * IC + ib * P:ic * IC + (ib + 1) * P]
                    .bitcast(F32))
        for cb in range(2):
            nc.sync.dma_start(
                o_out[cb * P:(cb + 1) * P, ic * IC:(ic + 1) * IC],
                ost[cb][:])

    groups = [(ic, grp) for ic in range(NIC) for grp in range(NGRP)]
    pending = None  # (ic, grp, ex) whose mm2 is not yet emitted
    for (ic, grp) in groups:
        ex = emit_mm1(ic, grp)
        if pending is not None:
            pic, pgrp, pex = pending
            emit_mm2(pic, pgrp, pex)
            if pgrp == NGRP - 1:
                emit_epilogue(pic)
        pending = (ic, grp, ex)
    pic, pgrp, pex = pending
    emit_mm2(pic, pgrp, pex)
    emit_epilogue(pic)


def _preprocess(inputs):
    """Fold conv bias + BN into effective weights/biases, per-core input maps."""
    f32 = np.float32
    q = np.ascontiguousarray(inputs["q"], dtype=f32)[..., 0]   # [B, 256, N]
    k = np.ascontiguousarray(inputs["k"], dtype=f32)[..., 0]   # [B, 128, N]

    def fold(W, b, scale, bias, mean, var):
        inv = (np.asarray(scale, f32) /
               np.sqrt(np.asarray(var, f32) + f32(EPS))).astype(f32)
        W_eff = (inv[:, None] * np.asarray(W, f32)).astype(f32)
        delta = ((np.asarray(b, f32) - np.asarray(mean, f32)) * inv
                 + np.asarray(bias, f32)).astype(f32)
        return W_eff, delta

    Wf_e, d_f = fold(inputs["Wf"], inputs["bf"], inputs["fs"], inputs["fb"],
                     inputs["fm"], inputs["fv"])
    Wg_e, d_g = fold(inputs["Wg"], inputs["bg"], inputs["gs"], inputs["gb"],
                     inputs["gm"], inputs["gv"])
    Wh_e, d_h = fold(inputs["Wh"], inputs["bh"], inputs["hs"], inputs["hb"],
                     inputs["hm"], inputs["hv"])

    gamma = f32(np.asarray(inputs["gamma"], f32).reshape(-1)[0])
    wfT = Wf_e.T                                   # [256, 64]
    wgT = Wg_e.T                                   # [128, 64]
    shared = {
        "wfd": _round_tf32(np.concatenate([wfT, wfT], axis=1)),  # [256, 128]
        "wgd": _round_tf32(np.concatenate([wgT, wgT], axis=1)),  # [128, 128]
        "whT": _round_tf32(Wh_e.T),                              # [128, 256]
        "dfd": np.tile(d_f.reshape(L, 1), (2, 1)),               # [128, 1]
        "dgd": np.tile(d_g.reshape(L, 1), (2, 1)),
        "dhbc": np.broadcast_to(d_h, (P, C1)).copy(),
        "gmb": np.full((P, 1), gamma, f32),
        "ident": _to_bf16(np.eye(P, dtype=f32)),
    }
    in_maps = []
    for b_ in range(B):
        m = dict(shared)
        m["qr"] = _round_tf32(q[b_])
        m["k2r"] = _round_tf32(k[b_])
        in_maps.append(m)
    return in_maps


def _get_nc():
    global _BUILT
    if _BUILT is None:
        _BUILT = _build()
    return _BUILT


def kernel(**inputs):
    nc = _get_nc()
    in_maps = _preprocess(inputs)
    res = run_bass_kernel_spmd(nc, in_maps, core_ids=list(range(B)))
    out = np.stack([res.results[i]["o_out"] for i in range(B)])
    return out[..., None].astype(np.float32)


if __name__ == "__main__":
    rng = np.random.default_rng(0)
    fake = {
        "q": rng.standard_normal((B, C1, N, 1), dtype=np.float32),
        "k": rng.standard_normal((B, C2, N, 1), dtype=np.float32),
        "Wf": rng.standard_normal((L, C1), dtype=np.float32) * 0.06,
        "bf": rng.standard_normal(L, dtype=np.float32) * 0.01,
        "fs": rng.random(L, dtype=np.float32) + 0.5,
        "fb": rng.standard_normal(L, dtype=np.float32) * 0.1,
        "fm": rng.standard_normal(L, dtype=np.float32) * 0.1,
        "fv": rng.random(L, dtype=np.float32) + 0.5,
        "Wg": rng.standard_normal((L, C2), dtype=np.float32) * 0.09,
        "bg": rng.standard_normal(L, dtype=np.float32) * 0.01,
        "gs": rng.random(L, dtype=np.float32) + 0.5,
        "gb": rng.standard_normal(L, dtype=np.float32) * 0.1,
        "gm": rng.standard_normal(L, dtype=np.float32) * 0.1,
        "gv": rng.random(L, dtype=np.float32) + 0.5,
        "Wh": rng.standard_normal((C1, C2), dtype=np.float32) * 0.09,
        "bh": rng.standard_normal(C1, dtype=np.float32) * 0.01,
        "hs": rng.random(C1, dtype=np.float32) + 0.5,
        "hb": rng.standard_normal(C1, dtype=np.float32) * 0.1,
        "hm": rng.standard_normal(C1, dtype=np.float32) * 0.1,
        "hv": rng.random(C1, dtype=np.float32) + 0.5,
        "gamma": np.array([-1.1], dtype=np.float32),
    }
    out = kernel(**fake)
    print("out", out.shape, out.dtype, float(np.abs(out).max()))
